# revision 1
# baseline (speedup 1.0000x reference)
"""Trainium2 Bass kernel for per-batch masked (fill->keep) attention.

Problem (hardcoded): B=8 batches, each batch = 2048 'fill' rows followed by
4096 'keep' rows, C_IN=256, C_KQ=64, C_OUT=256.
  q = fill @ Wq.T + bq;  k = keep @ Wk.T + bk;  v = keep @ Wv.T + bv
  out_fill = softmax(q k^T / 8) @ v ;  keep rows pass through.

Sharding: 1 batch per NeuronCore (8 cores, pure data parallel, no
collectives).

Per-core layout strategy:
  - PE-transpose features to featT [C_IN, rows] so projections can use the
    tensor engine directly.
  - qT [64, 2048] and kT [64, 4096] kept transposed; v [4096, 257] natural
    with a ones-column appended (col 256) so the softmax denominator comes
    out of the attn@v matmul for free.
  - scores are computed TRANSPOSED: sT[k_row, f_row] = kT.T @ qT, so
    exp(sT) tiles are directly the lhsT operand of attn@v (zero attention
    transposes).
  - softmax without max subtraction: scores ~ N(0,1) (max ~ 5), exp is safe
    in fp32 and the result is mathematically identical.
"""

import os
import sys

import numpy as np

sys.path.insert(0, "/opt/trn_rl_repo")

B, NF, NK = 8, 2048, 4096
CIN, CKQ, COUT = 256, 64, 256
R = NF + NK  # rows per batch/core

# matmul dtype mode: "f32" (safe, 4 cyc/row), "f32r" (1 cyc/row at free>=256),
# "bf16" (1 cyc/row, lower precision storage)
MM_DT_MODE = os.environ.get("MM_DT", "bf16")

_COMPILED = {}


def build_bass():
    import concourse.bass as bass
    import concourse.mybir as mybir
    import concourse.tile as tile
    from concourse import bacc
    from concourse.bass import ts
    from concourse.masks import make_identity

    f32 = mybir.dt.float32
    f32r = mybir.dt.float32r
    bf16 = mybir.dt.bfloat16
    Act = mybir.ActivationFunctionType

    if MM_DT_MODE == "bf16":
        store_dt = bf16
    elif MM_DT_MODE == "f32r":
        # data consumed by an FP32r matmul must be *written* as float32r
        # (engines round on write); bitcasting plain f32 is rejected by the
        # BIR verifier.
        store_dt = f32r
    else:
        store_dt = f32

    # Bacc (not plain Bass): its finalize() runs move_matmul_waits_to_ldweights
    # + generate_event_semaphores, required to satisfy the per-instruction
    # sync-wait limits of TRN2 codegen.
    nc = bacc.Bacc(None, target_bir_lowering=False)
    feat = nc.dram_tensor("features", [R, CIN], f32, kind="ExternalInput")
    wq_d = nc.dram_tensor("Wq", [CKQ, CIN], f32, kind="ExternalInput")
    bq_d = nc.dram_tensor("bq", [CKQ], f32, kind="ExternalInput")
    wk_d = nc.dram_tensor("Wk", [CKQ, CIN], f32, kind="ExternalInput")
    bk_d = nc.dram_tensor("bk", [CKQ], f32, kind="ExternalInput")
    wv_d = nc.dram_tensor("Wv", [COUT, CIN], f32, kind="ExternalInput")
    bv_d = nc.dram_tensor("bv", [COUT], f32, kind="ExternalInput")
    out = nc.dram_tensor("out", [R, CIN], f32, kind="ExternalOutput")

    REPEAT = int(os.environ.get("KREPEAT", "1"))
    RC = 512          # row chunk for load/transpose/project
    NCH = R // RC     # 12 chunks: 0-3 fill, 4-11 keep
    NKT = NK // 128   # 32 keep tiles of 128 rows
    FB = 512          # f-block (query block) for scores
    NFB = NF // FB    # 4

    with tile.TileContext(nc) as tc:
        with (
            tc.tile_pool(name="consts", bufs=1) as consts,
            tc.tile_pool(name="persist", bufs=1) as persist,
        ):
            # --- constants / weights prep ---
            identity = consts.tile([128, 128], store_dt)
            make_identity(nc, identity)
            identity_f32 = consts.tile([128, 128], f32)
            make_identity(nc, identity_f32)

            wq_nat = consts.tile([CKQ, CIN], f32)
            wk_nat = consts.tile([CKQ, CIN], f32)
            wv_nat = consts.tile([128, 2, CIN], f32)
            nc.sync.dma_start(out=wq_nat, in_=wq_d[:, :])
            nc.sync.dma_start(out=wk_nat, in_=wk_d[:, :])
            nc.sync.dma_start(
                out=wv_nat, in_=wv_d[:, :].rearrange("(t p) c -> p t c", p=128)
            )
            bq_sb = consts.tile([CKQ, 1], f32)
            bk_sb = consts.tile([CKQ, 1], f32)
            nc.sync.dma_start(out=bq_sb, in_=bq_d[:].unsqueeze(1))
            nc.sync.dma_start(out=bk_sb, in_=bk_d[:].unsqueeze(1))
            bv_bcast = consts.tile([128, COUT], f32)
            # cols appended to v: [1.0, 0.0] — ones give the softmax
            # denominator; the zero col pads the moving free dim to an even
            # 258 (f32r matmul ISA rejects odd free sizes).
            onz_sb = consts.tile([128, 2], f32)
            nc.vector.memset(onz_sb, 0.0)
            nc.vector.memset(onz_sb[:, 0:1], 1.0)
            bv_ap = bv_d[:]
            bv_b = bass.AP(
                tensor=bv_ap.tensor, offset=bv_ap.offset, ap=[[0, 128]] + bv_ap.ap
            )
            nc.sync.dma_start(out=bv_bcast, in_=bv_b)

            # transposed weights: [c_in (2x128 part), d]
            wqT = consts.tile([128, 2, CKQ], store_dt)
            wkT = consts.tile([128, 2, CKQ], store_dt)
            wvT = consts.tile([128, 2, COUT], store_dt)
            with tc.tile_pool(name="pwt", bufs=2, space="PSUM") as pwt:
                for s in range(2):
                    pt = pwt.tile([128, CKQ], f32, tag="wqk")
                    nc.tensor.transpose(
                        pt, wq_nat[:, ts(s, 128)], identity_f32[:CKQ, :CKQ]
                    )
                    nc.vector.tensor_copy(wqT[:, s, :], pt)
                    pt2 = pwt.tile([128, CKQ], f32, tag="wqk")
                    nc.tensor.transpose(
                        pt2, wk_nat[:, ts(s, 128)], identity_f32[:CKQ, :CKQ]
                    )
                    nc.vector.tensor_copy(wkT[:, s, :], pt2)
                    for t in range(2):
                        pt3 = pwt.tile([128, 128], f32, tag="wv")
                        nc.tensor.transpose(
                            pt3, wv_nat[:, t, ts(s, 128)], identity_f32
                        )
                        nc.vector.tensor_copy(wvT[:, s, ts(t, 128)], pt3)

            # --- persistent activations ---
            # qT lives duplicated on partitions 0-63 and 64-127 so the scores
            # matmuls can run 2 keep-tiles concurrently in 64x128 PE row-tiling
            # (T0 reads SBUF 0-63, T8 reads 64-127).
            qT_sb = persist.tile([128, NF], store_dt)
            # kT pair tiles: [2x64 partition halves (even/odd j), pair a, 128]
            kTp_tiles = [
                persist.tile(
                    [128, 2, 128], store_dt, tag=f"kTp{i}", name=f"kTp{i}"
                )
                for i in range(8)
            ]
            v_all = persist.tile([128, NKT, COUT + 2], store_dt)

            # --- phase A/B: load, transpose, project, passthrough ---
            # All PSUM pools are co-resident (8 banks total) so phase C
            # scores can start while phase A/B is still streaming: no bank
            # aliasing between phases, hence no false cross-phase deps.
            with (
                tc.tile_pool(name="fnat", bufs=12) as fpool,
                tc.tile_pool(name="fT", bufs=6) as ftpool,
                tc.tile_pool(name="pab", bufs=2, space="PSUM") as pab,
                tc.tile_pool(name="etile", bufs=3) as epool,
                tc.tile_pool(name="osb", bufs=6) as opool,
                tc.tile_pool(name="small", bufs=4) as spool,
                tc.tile_pool(name="pscore", bufs=2, space="PSUM") as pscore,
                tc.tile_pool(name="pout", bufs=2, space="PSUM") as pout,
            ):
                for _rep in range(REPEAT):
                  for rc in range(NCH):
                      fnat = fpool.tile(
                          [128, 4, CIN], store_dt, tag="fnat"
                      )
                      rows = feat[ts(rc, RC), :]
                      # SWDGE casts f32 -> bf16 in flight; transposes then run
                      # at 1 cycle/row instead of 2
                      nc.gpsimd.dma_start(
                          out=fnat, in_=rows.rearrange("(t p) c -> p t c", p=128)
                      )
                      is_fill = rc < NF // RC
                      fT = ftpool.tile([128, 2, RC], store_dt, tag="fT")
                      for s in range(2):
                          tp = pab.tile([128, 4, 128], store_dt, tag="ab", name="tp")
                          for t in range(4):
                              nc.tensor.transpose(
                                  tp[:, t, :], fnat[:, t, ts(s, 128)], identity
                              )
                          eng = nc.scalar if s == 0 else nc.vector
                          if s == 0:
                              nc.scalar.copy(
                                  fT[:, s, :],
                                  tp.rearrange("p t c -> p (t c)"),
                              )
                          else:
                              nc.vector.tensor_copy(
                                  fT[:, s, :],
                                  tp.rearrange("p t c -> p (t c)"),
                              )
                      if is_fill:
                          qp = pab.tile([CKQ, RC], f32, tag="ab", name="qp")
                          nc.tensor.matmul(
                              qp, wqT[:, 0, :], fT[:, 0, :],
                              start=True, stop=False,
                          )
                          nc.tensor.matmul(
                              qp, wqT[:, 1, :], fT[:, 1, :],
                              start=False, stop=True,
                          )
                          nc.vector.tensor_scalar_add(
                              qT_sb[0:CKQ, ts(rc, RC)], qp, bq_sb
                          )
                          nc.vector.tensor_scalar_add(
                              qT_sb[CKQ:128, ts(rc, RC)], qp, bq_sb
                          )
                      else:
                          kc = rc - NF // RC
                          kp = pab.tile([CKQ, RC], f32, tag="ab", name="kp")
                          nc.tensor.matmul(
                              kp, wkT[:, 0, :], fT[:, 0, :],
                              start=True, stop=False,
                          )
                          nc.tensor.matmul(
                              kp, wkT[:, 1, :], fT[:, 1, :],
                              start=False, stop=True,
                          )
                          kp_r = kp.rearrange("p (a q c) -> p a q c", a=2, q=2)
                          nc.vector.tensor_scalar_add(
                              kTp_tiles[kc][0:CKQ, :, :], kp_r[:, :, 0, :], bk_sb
                          )
                          nc.vector.tensor_scalar_add(
                              kTp_tiles[kc][CKQ:128, :, :], kp_r[:, :, 1, :], bk_sb
                          )
                          for t in range(4):
                              j = kc * 4 + t
                              vp = pab.tile([128, COUT], f32, tag="ab", name="vp")
                              nc.tensor.matmul(
                                  vp, fT[:, 0, ts(t, 128)], wvT[:, 0, :],
                                  start=True, stop=False,
                              )
                              nc.tensor.matmul(
                                  vp, fT[:, 1, ts(t, 128)], wvT[:, 1, :],
                                  start=False, stop=True,
                              )
                              nc.vector.tensor_add(
                                  v_all[:, j, :COUT], vp, bv_bcast
                              )

                  # ones/zero pad columns for every keep tile, one
                  # broadcast DMA (issued here so identity owns Pool at t=0)
                  if _rep == 0:
                      onz_b = bass.AP(
                          tensor=onz_sb.tensor, offset=onz_sb.offset,
                          ap=[onz_sb.ap[0], [0, NKT], onz_sb.ap[1]],
                      )
                      nc.gpsimd.dma_start(
                          out=v_all[:, :, COUT : COUT + 2], in_=onz_b
                      )
                  # --- phase C: scoresT -> exp -> attn@v -> divide ---
                  for fb in range(NFB):
                      e_pairs = []
                      for m in range(NKT // 2):
                          kc, a = m // 2, m % 2
                          spp = pscore.tile([128, 2, FB], f32, tag="sp")
                          nc.tensor.matmul(
                              spp[:, 0, :],
                              kTp_tiles[kc][0:CKQ, a, :],
                              qT_sb[0:CKQ, ts(fb, FB)],
                              start=True, stop=True, tile_position=(0, 0),
                          )
                          nc.tensor.matmul(
                              spp[:, 1, :],
                              kTp_tiles[kc][CKQ:128, a, :],
                              qT_sb[CKQ:128, ts(fb, FB)],
                              start=True, stop=True, tile_position=(64, 0),
                          )
                          ep = epool.tile(
                              [128, 2, FB], store_dt, tag=f"ep{m}", name=f"ep{m}"
                          )
                          nc.scalar.activation(ep, spp, Act.Exp, scale=0.125)
                          e_pairs.append(ep)
                      for fs in range(FB // 128):
                          op = pout.tile([128, COUT + 2], f32, tag="op")
                          for j in range(NKT):
                              nc.tensor.matmul(
                                  op,
                                  e_pairs[j // 2][:, j % 2, ts(fs, 128)],
                                  v_all[:, j, :],
                                  start=(j == 0), stop=(j == NKT - 1),
                              )
                          rec = spool.tile([128, 1], f32, tag="rec")
                          nc.vector.reciprocal(rec, op[:, COUT : COUT + 1])
                          ob = opool.tile([128, COUT], f32, tag="ob")
                          nc.vector.tensor_scalar_mul(ob, op[:, :COUT], rec)
                          nc.scalar.dma_start(
                              out=out[fb * FB + fs * 128 : fb * FB + (fs + 1) * 128, :],
                              in_=ob,
                          )
                # keep rows pass through unchanged: DRAM->DRAM copies, issued
                # late so the input DMA stream owns the bandwidth up front
                for rc in range(NF // RC, NCH):
                    nc.scalar.dma_start(
                        out=out[ts(rc, RC), :], in_=feat[ts(rc, RC), :]
                    )
    nc.finalize()
    return nc


def get_nc():
    if "nc" not in _COMPILED:
        _COMPILED["nc"] = build_bass()
    return _COMPILED["nc"]


def kernel(**inputs):
    from concourse.bass_utils import run_bass_kernel_spmd

    nc = get_nc()
    features = np.ascontiguousarray(inputs["features"], dtype=np.float32)
    fb = features.reshape(B, R, CIN)
    common = {
        k: np.ascontiguousarray(inputs[k], dtype=np.float32)
        for k in ("Wq", "bq", "Wk", "bk", "Wv", "bv")
    }
    in_maps = [{"features": fb[b], **common} for b in range(B)]
    res = run_bass_kernel_spmd(nc, in_maps, core_ids=list(range(B)))
    outs = [res.results[b]["out"] for b in range(B)]
    return np.concatenate(outs, axis=0).reshape(B * R, COUT).astype(np.float32)

